# revision 20
# baseline (speedup 1.0000x reference)
"""ChebNet (K=4, two ChebConv layers + ReLU) on 8 Trainium2 NeuronCores.

Strategy (graph-partitioned SpMM, memory-regime):
 - Host: compute sym-norm edge weights w_norm and diag of L_hat; relabel nodes
   by in-degree; deal 128-node bands round-robin to the 8 cores; build a
   padded ELL structure per core (slots per dst node), split into two gather
   calls (int16 index limit 32768 -> two base offsets into the node array).
 - Clenshaw evaluation of sum_k T_k(L_hat) @ (x W_k): only 3 SpMM props per
   layer at the layer's *output* width (64 / 40-padded-to-64) instead of the
   input width.
 - Device per prop: dma_gather rows of the AllGathered vector from HBM into
   SBUF ELL tiles [128, K, 64], multiply by per-slot weights (in-place DVE),
   reduce over slots, fuse Clenshaw AXPYs (scalar_tensor_tensor), u_k terms
   computed on the fly on the PE from resident xT / hT.
 - Cross-core: one AllGather (shared-output) per prop carries the new
   Chebyshev vector to every core's HBM for the next gather.
"""
import os
import numpy as np

N, E, F, H, C, KCH = 50000, 1600000, 128, 64, 40, 4
NCORES, P = 8, 128
IDX_CAP = 32768
BANDS = 49                 # ceil(ceil(N/128)/8)
SLOTS = BANDS * P          # 6272 node slots per core
NP = NCORES * SLOTS        # 50176 padded global node slots
BASE_B = NP - IDX_CAP      # 17408

LAST_RESULTS = {}          # test harness introspection (timing/trace)
DEBUG_STAGE = None         # when set (e.g. "b2"), program dumps that stage to "dbg"
STOP_AFTER = None          # when set, truncate the program after that stage
DEBUG_SKIP = set()         # subset of {"gather", "mac", "stt", "u"} to stub out


# ----------------------------------------------------------------- host plan

def _build_plan(edge_index, edge_weight):
    src = np.asarray(edge_index[0]).astype(np.int64)
    dst = np.asarray(edge_index[1]).astype(np.int64)
    w = np.asarray(edge_weight, np.float64)

    deg = np.zeros(N, np.float64)
    np.add.at(deg, src, w)
    dis = np.where(deg > 0, 1.0 / np.sqrt(np.maximum(deg, 1e-12)), 0.0)
    w_norm = (-dis[src] * w * dis[dst]).astype(np.float32)
    diag_old = np.where(deg > 0, 0.0, -1.0).astype(np.float32)

    # nodes with deg_out == 0 contribute via the separate diag term on device
    indeg = np.bincount(dst, minlength=N)

    # relabel: degree-sorted band deal.  node old -> (core, pos, r)
    order = np.argsort(-indeg, kind="stable")
    rk = np.arange(N) // P                  # band rank of sorted position
    core_arr = rk % NCORES
    pos_arr = rk // NCORES
    r_arr = np.arange(N) % P
    new_id = np.empty(N, np.int64)
    # padded id = core*SLOTS + r*BANDS + pos   (matches [128, BANDS, 64] layout)
    new_id[order] = core_arr * SLOTS + r_arr * BANDS + pos_arr

    src_n = new_id[src]
    dst_n = new_id[dst]

    core_of = dst_n // SLOTS
    remv = dst_n % SLOTS
    r_of = remv // BANDS
    pos_of = remv % BANDS

    # three gather bases: O0=0, O1, O2; every src eligible for >=1 call
    O1 = (NP - IDX_CAP) // 2
    O2 = NP - IDX_CAP
    NB = 3
    kk = np.zeros((NB, NCORES, BANDS, P), np.int32)
    per_core = []
    for c in range(NCORES):
        m = core_of == c
        es, ew = src_n[m], w_norm[m]
        pp, rr = pos_of[m], r_of[m]
        loc = pp * P + rr
        o = np.argsort(loc, kind="stable")
        es, ew, loc = es[o], ew[o], loc[o]
        starts = np.searchsorted(loc, np.arange(BANDS * P))

        # class by eligibility: 0:[0,O1) only0, 1:[O1,O2) 0/1, 2:[O2,IDX_CAP) any,
        # 3:[IDX_CAP, O1+IDX_CAP) 1/2, 4:[O1+IDX_CAP, NP) only2
        cls = np.digitize(es, [O1, O2, IDX_CAP, O1 + IDX_CAP])
        cnt = np.stack([np.bincount(loc[cls == i], minlength=BANDS * P)
                        for i in range(5)]).astype(np.int32)
        degl = cnt.sum(0)
        t = (degl + 2) // 3
        k0 = np.clip(t, cnt[0], cnt[0] + cnt[1] + cnt[2])
        used2_0 = np.maximum(0, k0 - cnt[0] - cnt[1])
        k2 = np.clip(np.minimum(t, degl - k0), cnt[4],
                     cnt[4] + cnt[3] + (cnt[2] - used2_0))
        k1 = degl - k0 - k2
        kk[0, c] = k0.reshape(BANDS, P)
        kk[1, c] = k1.reshape(BANDS, P)
        kk[2, c] = k2.reshape(BANDS, P)

        o2 = np.lexsort((cls, loc))
        es, ew, loc = es[o2], ew[o2], loc[o2]
        rank = np.arange(es.size) - starts[loc]
        call = np.where(rank < k0[loc], 0, np.where(rank < (k0 + k1)[loc], 1, 2))
        base = np.array([0, O1, O2])[call]
        assert (es - base >= 0).all() and (es - base < IDX_CAP).all()
        slot = rank - np.where(call == 0, 0, np.where(call == 1, k0[loc], (k0 + k1)[loc]))
        per_core.append((es, ew, loc, call, slot))

    Ks = kk.max(axis=(1, 3))                  # [3, BANDS]
    offs = np.concatenate([np.zeros((NB, 1), np.int64),
                           np.cumsum(Ks, axis=1)], axis=1)
    sumKs = Ks.sum(axis=1).astype(np.int64)

    idxs = [np.zeros((NCORES, BANDS * P, int(Ks[i].max())), np.int32) for i in range(NB)]
    wvs = [np.zeros((NCORES, BANDS * P, int(Ks[i].max())), np.float32) for i in range(NB)]
    bases = [0, O1, O2]
    for c in range(NCORES):
        es, ew, loc, call, slot = per_core[c]
        for i in range(NB):
            m = call == i
            idxs[i][c, loc[m], slot[m]] = es[m] - bases[i]
            wvs[i][c, loc[m], slot[m]] = ew[m]

    # device-layout tiles
    def pack_idx(idx, Karr):
        tiles = np.zeros((NCORES, P, 8 * int(np.sum(Karr))), np.int16)
        for c in range(NCORES):
            cols = []
            a = idx[c].reshape(BANDS, P, -1)
            for pos in range(BANDS):
                kq = int(Karr[pos])
                lst = a[pos, :, :kq].T.reshape(-1)       # j = k*128 + r
                cols.append(np.tile(lst.reshape(-1, 16).T, (8, 1)))
            tiles[c] = np.concatenate(cols, axis=1).astype(np.int16)
        return tiles

    def pack_w(wv, Karr):
        tiles = np.zeros((NCORES, P, int(np.sum(Karr))), np.float32)
        for c in range(NCORES):
            a = wv[c].reshape(BANDS, P, -1)
            col = 0
            for pos in range(BANDS):
                kq = int(Karr[pos])
                tiles[c][:, col:col + kq] = a[pos, :, :kq]
                col += kq
        return tiles

    idx_t = [pack_idx(idxs[i], Ks[i]) for i in range(NB)]
    w_t = [pack_w(wvs[i], Ks[i]) for i in range(NB)]

    diag_t = np.zeros((NCORES, P, BANDS), np.float32)
    dn = np.zeros(NP, np.float32)
    dn[new_id] = diag_old
    diag_t[:] = dn.reshape(NCORES, P, BANDS)

    return dict(new_id=new_id, Ks=Ks, offs=offs, sumKs=sumKs, bases=bases,
                idx=idx_t, w=w_t, diag=diag_t)


# --------------------------------------------------------------- device prog

def _build_program(Ks, offs, sumKs, bases):
    import concourse.bacc as bacc
    import concourse.bass as bass
    import concourse.mybir as mybir
    import concourse.tile as tile
    from concourse.masks import make_identity

    f32 = mybir.dt.float32
    i16 = mybir.dt.int16
    ADD = mybir.AluOpType.add
    SUB = mybir.AluOpType.subtract
    MULT = mybir.AluOpType.mult
    AXX = mybir.AxisListType.X

    nc = bacc.Bacc(num_devices=NCORES, target_bir_lowering=False)

    xT_in = nc.dram_tensor("xT", [P, SLOTS], f32, kind="ExternalInput")
    W1r_in = nc.dram_tensor("W1r", [P, 4 * H], f32, kind="ExternalInput")
    W2r_in = nc.dram_tensor("W2r", [H, 4 * C], f32, kind="ExternalInput")
    b1_in = nc.dram_tensor("bias1", [P, H], f32, kind="ExternalInput")
    b2_in = nc.dram_tensor("bias2", [P, C], f32, kind="ExternalInput")
    idx_ins = [nc.dram_tensor(f"idx{i}", [P, 8 * int(sumKs[i])], i16,
                              kind="ExternalInput") for i in range(3)]
    w_ins = [nc.dram_tensor(f"w{i}", [P, int(sumKs[i])], f32,
                            kind="ExternalInput") for i in range(3)]
    diag_in = nc.dram_tensor("diag", [P, BANDS], f32, kind="ExternalInput")
    out_ext = nc.dram_tensor("out", [P, BANDS * C], f32, kind="ExternalOutput")

    vcur = nc.dram_tensor("vcur", [NP, H], f32, addr_space="Shared")
    ybounce = nc.dram_tensor("ybounce", [P, BANDS * H], f32)
    dbg = (nc.dram_tensor("dbg", [P, BANDS * H], f32, kind="ExternalOutput")
           if DEBUG_STAGE else None)

    RG = [list(range(NCORES))]

    with tile.TileContext(nc) as tc:
        with (
            tc.tile_pool(name="const", bufs=1) as cp,
            tc.tile_pool(name="work", bufs=2) as wp,
            tc.tile_pool(name="small", bufs=4) as sp,
            tc.tile_pool(name="psum", bufs=2, space="PSUM") as pp,
        ):
            # ---- resident loads
            xT = cp.tile([P, SLOTS], f32)
            nc.sync.dma_start(xT[:], xT_in[:])
            W1r = cp.tile([P, 4 * H], f32)
            nc.sync.dma_start(W1r[:], W1r_in[:])
            W2r = cp.tile([H, 4 * C], f32)
            nc.sync.dma_start(W2r[:], W2r_in[:])
            bias1 = cp.tile([P, H], f32)
            nc.sync.dma_start(bias1[:], b1_in[:])
            bias2 = cp.tile([P, C], f32)
            nc.sync.dma_start(bias2[:], b2_in[:])
            idx_ts, w_ts = [], []
            for i in range(3):
                it = cp.tile([P, 8 * int(sumKs[i])], i16, tag=f"idx{i}")
                nc.sync.dma_start(it[:], idx_ins[i][:])
                idx_ts.append(it)
                wt = cp.tile([P, int(sumKs[i])], f32, tag=f"w{i}")
                nc.sync.dma_start(wt[:], w_ins[i][:])
                w_ts.append(wt)
            diag = cp.tile([P, BANDS], f32)
            nc.sync.dma_start(diag[:], diag_in[:])
            ident = cp.tile([P, P], f32)
            make_identity(nc, ident)

            b0 = cp.tile([P, BANDS * H], f32, tag="b0")
            b1t = cp.tile([P, BANDS * H], f32, tag="b1")
            b2t = cp.tile([P, BANDS * H], f32, tag="b2")
            hT = cp.tile([H, SLOTS], f32, tag="hT")
            outb = cp.tile([P, BANDS * C], f32, tag="outb")
            nc.vector.memset(outb[:], 0.0)

            def bsl(t, pos, dd=H):
                return t[:, pos * H:pos * H + dd]

            def u_mm(pos, layer, k, dd):
                """u_k band on PSUM: layer 1 from xT/W1r, layer 2 from hT/W2r."""
                ups = pp.tile([P, dd], f32, tag="u", space="PSUM")
                if layer == 1:
                    nc.tensor.matmul(
                        ups[:], lhsT=xT[:, pos * P:(pos + 1) * P],
                        rhs=W1r[:, k * H:(k + 1) * H], start=True, stop=True)
                else:
                    nc.tensor.matmul(
                        ups[:], lhsT=hT[:, pos * P:(pos + 1) * P],
                        rhs=W2r[:, k * C:(k + 1) * C], start=True, stop=True)
                return ups

            def publish(bsrc):
                """b buffer -> ybounce -> AllGather -> vcur."""
                nc.sync.dma_start(ybounce[:], bsrc[:])
                nc.gpsimd.collective_compute(
                    "AllGather", mybir.AluOpType.bypass, replica_groups=RG,
                    ins=[ybounce[:].opt()], outs=[vcur[:].opt()])

            def spmm_y(pos, dd):
                """y = scatter-part of (L_hat @ v) for band pos; returns [P, dd]."""
                if "gather" in DEBUG_SKIP:
                    y = sp.tile([P, dd], f32, tag="y")
                    nc.vector.memset(y[:], 0.0)
                    return y
                accs = []
                for i in range(3):
                    kq = int(Ks[i][pos])
                    if kq == 0:
                        continue
                    g = wp.tile([P, kq, H], f32, tag=f"g{i}")
                    nc.gpsimd.dma_gather(
                        out_ap=g[:], in_ap=vcur[bases[i]:bases[i] + IDX_CAP, :],
                        idxs_ap=idx_ts[i][:, 8 * int(offs[i][pos]):
                                          8 * int(offs[i][pos] + kq)],
                        num_idxs=P * kq, num_idxs_reg=P * kq, elem_size=H,
                        single_packet=False)
                    if "mac" in DEBUG_SKIP:
                        continue
                    nc.vector.tensor_tensor(
                        g[:, :, :dd], g[:, :, :dd],
                        w_ts[i][:, int(offs[i][pos]):int(offs[i][pos] + kq)]
                        .unsqueeze(2).to_broadcast([P, kq, dd]), op=MULT)
                    acc = sp.tile([P, dd], f32, tag=f"acc{i}")
                    nc.vector.tensor_reduce(
                        acc[:], g[:, :, :dd].transpose([0, 2, 1]), axis=AXX, op=ADD)
                    accs.append(acc)
                y = sp.tile([P, dd], f32, tag="y")
                if "mac" in DEBUG_SKIP or not accs:
                    nc.vector.memset(y[:], 0.0)
                    return y
                if len(accs) == 1:
                    return accs[0]
                nc.vector.tensor_add(y[:], accs[0][:], accs[1][:])
                for a in accs[2:]:
                    nc.vector.tensor_add(y[:], y[:], a[:])
                return y

            def prop_phase(mode, layer, k, dd, bv, bdst, bprev2):
                """One Clenshaw prop: bdst = 2(L v) + u_k [- bprev2]  or the
                final combine (mode 'fin')."""
                for pos in range(BANDS):
                    y = spmm_y(pos, dd)
                    if "stt" in DEBUG_SKIP:
                        t = y
                    else:
                        t = sp.tile([P, dd], f32, tag="t")
                        nc.vector.scalar_tensor_tensor(
                            out=t[:], in0=bsl(bv, pos, dd), scalar=diag[:, pos:pos + 1],
                            in1=y[:], op0=MULT, op1=ADD)
                    if "u" in DEBUG_SKIP:
                        ups = sp.tile([P, dd], f32, tag="ustub")
                        nc.vector.memset(ups[:], 0.0)
                    else:
                        ups = u_mm(pos, layer, k, dd)
                    if mode == "b":          # 2t + u [- bprev2]
                        s = sp.tile([P, dd], f32, tag="s")
                        nc.vector.scalar_tensor_tensor(
                            out=s[:], in0=t[:], scalar=2.0, in1=ups[:],
                            op0=MULT, op1=ADD)
                        if bprev2 is not None:
                            nc.vector.tensor_sub(
                                bsl(bdst, pos, dd), s[:], bsl(bprev2, pos, dd))
                        else:
                            nc.vector.tensor_copy(bsl(bdst, pos, dd), s[:])
                    else:                    # fin: t - bprev2 + u + bias
                        s = sp.tile([P, dd], f32, tag="s")
                        nc.vector.tensor_sub(s[:], t[:], bsl(bprev2, pos, dd))
                        nc.vector.tensor_add(s[:], s[:], ups[:])
                        if layer == 1:
                            nc.vector.tensor_add(s[:], s[:], bias1[:, :dd])
                            h = bsl(bdst, pos, dd)
                            nc.vector.tensor_relu(h, s[:])
                            trp = pp.tile([H, P], f32, tag="tr", space="PSUM")
                            nc.tensor.transpose(out=trp[:], in_=h, identity=ident[:])
                            nc.scalar.copy(hT[:, pos * P:(pos + 1) * P], trp[:])
                        else:
                            nc.vector.tensor_add(s[:], s[:], bias2[:, :dd])
                            nc.vector.tensor_copy(
                                outb[:, pos * C:pos * C + C], s[:])

            def dump(stage, buf):
                if DEBUG_STAGE == stage:
                    nc.sync.dma_start(dbg[:], buf[:])

            def zero_tails():
                # zero the 40:64 columns of the b buffers for the narrow layer
                for bb in (b0, b1t, b2t):
                    nc.vector.memset(
                        bb[:].rearrange("p (b h) -> p b h", h=H)[:, :, C:H], 0.0)

            def u_loop(layer, k, dd, bdst):
                for pos in range(BANDS):
                    ups = u_mm(pos, layer, k, dd)
                    nc.vector.tensor_copy(bsl(bdst, pos, dd), ups[:])

            stages = [
                # ---------------- layer 1 ----------------
                ("u3", lambda: u_loop(1, 3, H, b0)),           # b3 = u3
                ("pub0", lambda: (publish(b0), dump("b3", b0))),
                ("b2", lambda: prop_phase("b", 1, 2, H, bv=b0, bdst=b1t,
                                          bprev2=None)),       # b2 = 2Lb3+u2
                ("pub1", lambda: (publish(b1t), dump("b2", b1t))),
                ("b1", lambda: prop_phase("b", 1, 1, H, bv=b1t, bdst=b2t,
                                          bprev2=b0)),         # b1 = 2Lb2-b3+u1
                ("pub2", lambda: (publish(b2t), dump("b1", b2t))),
                ("h", lambda: (prop_phase("fin", 1, 0, H, bv=b2t, bdst=b0,
                                          bprev2=b1t), dump("h", b0))),
                # ---------------- layer 2 ----------------
                ("zt", zero_tails),
                ("u3p", lambda: u_loop(2, 3, C, b1t)),         # b3' = u3'
                ("pub3", lambda: (publish(b1t), dump("u3p", b1t))),
                ("b2p", lambda: prop_phase("b", 2, 2, C, bv=b1t, bdst=b2t,
                                           bprev2=None)),
                ("pub4", lambda: (publish(b2t), dump("b2p", b2t))),
                ("b1p", lambda: prop_phase("b", 2, 1, C, bv=b2t, bdst=b0,
                                           bprev2=b1t)),
                ("pub5", lambda: (publish(b0), dump("b1p", b0))),
                ("fin", lambda: prop_phase("fin", 2, 0, C, bv=b0, bdst=None,
                                           bprev2=b2t)),
            ]
            for name, thunk in stages:
                thunk()
                if STOP_AFTER == name:
                    break

            nc.sync.dma_start(out_ext[:], outb[:])

    nc.compile()
    return nc


# -------------------------------------------------------------------- kernel

def kernel(x, edge_index, edge_weight, W1, b1, W2, b2):
    from concourse.bass_utils import run_bass_kernel_spmd

    x = np.asarray(x, np.float32)
    W1 = np.asarray(W1, np.float32)
    W2 = np.asarray(W2, np.float32)
    b1 = np.asarray(b1, np.float32)
    b2 = np.asarray(b2, np.float32)

    plan = _build_plan(edge_index, edge_weight)
    new_id = plan["new_id"]

    nc = _build_program(plan["Ks"], plan["offs"], plan["sumKs"], plan["bases"])

    # xT per core: [128 features, SLOTS] with node (pos, r) at column pos*128+r
    xp = np.zeros((NP, F), np.float32)
    xp[new_id] = x
    # padded id = c*SLOTS + r*BANDS + pos ; column order wanted: pos*128 + r
    xc = xp.reshape(NCORES, P, BANDS, F)          # [c, r, pos, F]
    xT_cores = np.ascontiguousarray(
        xc.transpose(0, 3, 2, 1).reshape(NCORES, F, SLOTS))  # [c, F, pos*128+r]

    W1r = np.ascontiguousarray(
        np.concatenate([W1[k * F:(k + 1) * F, :] for k in range(KCH)], axis=1))
    W2r = np.ascontiguousarray(
        np.concatenate([W2[k * H:(k + 1) * H, :] for k in range(KCH)], axis=1))
    bias1 = np.tile(b1[None, :], (P, 1)).astype(np.float32)
    bias2 = np.tile(b2[None, :], (P, 1)).astype(np.float32)

    in_maps = []
    for c in range(NCORES):
        in_maps.append({
            "xT": xT_cores[c],
            "W1r": W1r, "W2r": W2r, "bias1": bias1, "bias2": bias2,
            "idx0": plan["idx"][0][c], "idx1": plan["idx"][1][c],
            "idx2": plan["idx"][2][c],
            "w0": plan["w"][0][c], "w1": plan["w"][1][c], "w2": plan["w"][2][c],
            "diag": plan["diag"][c],
        })

    trace = bool(int(os.environ.get("CHEB_TRACE", "0")))
    import time as _time
    _t0 = _time.time()
    res = run_bass_kernel_spmd(nc, in_maps, core_ids=list(range(NCORES)),
                               trace=trace)
    LAST_RESULTS["res"] = res
    LAST_RESULTS["exec_wall_s"] = _time.time() - _t0

    outs = np.stack([np.asarray(res.results[c]["out"]) for c in range(NCORES)])
    # out tile [128, BANDS*C]: row r, cols pos*C.. ; padded id = c*SLOTS+r*BANDS+pos
    res_pad = outs.reshape(NCORES, P, BANDS, C).reshape(NP, C)
    return res_pad[new_id].astype(np.float32)


# revision 21
# speedup vs baseline: 1.1134x; 1.1134x over previous
"""ChebNet (K=4, two ChebConv layers + ReLU) on 8 Trainium2 NeuronCores.

Strategy (graph-partitioned SpMM, memory-regime):
 - Host: compute sym-norm edge weights w_norm and diag of L_hat; relabel nodes
   by in-degree; deal 128-node bands round-robin to the 8 cores; build a
   padded ELL structure per core (slots per dst node), split into two gather
   calls (int16 index limit 32768 -> two base offsets into the node array).
 - Clenshaw evaluation of sum_k T_k(L_hat) @ (x W_k): only 3 SpMM props per
   layer at the layer's *output* width (64 / 40-padded-to-64) instead of the
   input width.
 - Device per prop: dma_gather rows of the AllGathered vector from HBM into
   SBUF ELL tiles [128, K, 64], multiply by per-slot weights (in-place DVE),
   reduce over slots, fuse Clenshaw AXPYs (scalar_tensor_tensor), u_k terms
   computed on the fly on the PE from resident xT / hT.
 - Cross-core: one AllGather (shared-output) per prop carries the new
   Chebyshev vector to every core's HBM for the next gather.
"""
import os
import numpy as np

N, E, F, H, C, KCH = 50000, 1600000, 128, 64, 40, 4
NCORES, P = 8, 128
IDX_CAP = 32768
BANDS = 49                 # ceil(ceil(N/128)/8)
SLOTS = BANDS * P          # 6272 node slots per core
NP = NCORES * SLOTS        # 50176 padded global node slots
BASE_B = NP - IDX_CAP      # 17408

LAST_RESULTS = {}          # test harness introspection (timing/trace)
DEBUG_STAGE = None         # when set (e.g. "b2"), program dumps that stage to "dbg"
STOP_AFTER = None          # when set, truncate the program after that stage
DEBUG_SKIP = set()         # subset of {"gather", "mac", "stt", "u"} to stub out


# ----------------------------------------------------------------- host plan

def _build_plan(edge_index, edge_weight):
    src = np.asarray(edge_index[0]).astype(np.int64)
    dst = np.asarray(edge_index[1]).astype(np.int64)
    w = np.asarray(edge_weight, np.float64)

    deg = np.zeros(N, np.float64)
    np.add.at(deg, src, w)
    dis = np.where(deg > 0, 1.0 / np.sqrt(np.maximum(deg, 1e-12)), 0.0)
    w_norm = (-dis[src] * w * dis[dst]).astype(np.float32)
    diag_old = np.where(deg > 0, 0.0, -1.0).astype(np.float32)

    # nodes with deg_out == 0 contribute via the separate diag term on device
    indeg = np.bincount(dst, minlength=N)

    # relabel: degree-sorted band deal.  node old -> (core, pos, r)
    order = np.argsort(-indeg, kind="stable")
    rk = np.arange(N) // P                  # band rank of sorted position
    core_arr = rk % NCORES
    pos_arr = rk // NCORES
    r_arr = np.arange(N) % P
    new_id = np.empty(N, np.int64)
    # padded id = core*SLOTS + r*BANDS + pos   (matches [128, BANDS, 64] layout)
    new_id[order] = core_arr * SLOTS + r_arr * BANDS + pos_arr

    src_n = new_id[src]
    dst_n = new_id[dst]

    core_of = dst_n // SLOTS
    remv = dst_n % SLOTS
    r_of = remv // BANDS
    pos_of = remv % BANDS

    # three gather bases: O0=0, O1, O2; every src eligible for >=1 call
    O1 = (NP - IDX_CAP) // 2
    O2 = NP - IDX_CAP
    NB = 3
    kk = np.zeros((NB, NCORES, BANDS, P), np.int32)
    per_core = []
    for c in range(NCORES):
        m = core_of == c
        es, ew = src_n[m], w_norm[m]
        pp, rr = pos_of[m], r_of[m]
        loc = pp * P + rr
        o = np.argsort(loc, kind="stable")
        es, ew, loc = es[o], ew[o], loc[o]
        starts = np.searchsorted(loc, np.arange(BANDS * P))

        # class by eligibility: 0:[0,O1) only0, 1:[O1,O2) 0/1, 2:[O2,IDX_CAP) any,
        # 3:[IDX_CAP, O1+IDX_CAP) 1/2, 4:[O1+IDX_CAP, NP) only2
        cls = np.digitize(es, [O1, O2, IDX_CAP, O1 + IDX_CAP])
        cnt = np.stack([np.bincount(loc[cls == i], minlength=BANDS * P)
                        for i in range(5)]).astype(np.int32)
        degl = cnt.sum(0)
        t = (degl + 2) // 3
        k0 = np.clip(t, cnt[0], cnt[0] + cnt[1] + cnt[2])
        used2_0 = np.maximum(0, k0 - cnt[0] - cnt[1])
        k2 = np.clip(np.minimum(t, degl - k0), cnt[4],
                     cnt[4] + cnt[3] + (cnt[2] - used2_0))
        k1 = degl - k0 - k2
        kk[0, c] = k0.reshape(BANDS, P)
        kk[1, c] = k1.reshape(BANDS, P)
        kk[2, c] = k2.reshape(BANDS, P)

        o2 = np.lexsort((cls, loc))
        es, ew, loc = es[o2], ew[o2], loc[o2]
        rank = np.arange(es.size) - starts[loc]
        call = np.where(rank < k0[loc], 0, np.where(rank < (k0 + k1)[loc], 1, 2))
        base = np.array([0, O1, O2])[call]
        assert (es - base >= 0).all() and (es - base < IDX_CAP).all()
        slot = rank - np.where(call == 0, 0, np.where(call == 1, k0[loc], (k0 + k1)[loc]))
        per_core.append((es, ew, loc, call, slot))

    Ks = kk.max(axis=(1, 3))                  # [3, BANDS]
    offs = np.concatenate([np.zeros((NB, 1), np.int64),
                           np.cumsum(Ks, axis=1)], axis=1)
    sumKs = Ks.sum(axis=1).astype(np.int64)

    idxs = [np.zeros((NCORES, BANDS * P, int(Ks[i].max())), np.int32) for i in range(NB)]
    wvs = [np.zeros((NCORES, BANDS * P, int(Ks[i].max())), np.float32) for i in range(NB)]
    bases = [0, O1, O2]
    for c in range(NCORES):
        es, ew, loc, call, slot = per_core[c]
        for i in range(NB):
            m = call == i
            idxs[i][c, loc[m], slot[m]] = es[m] - bases[i]
            wvs[i][c, loc[m], slot[m]] = ew[m]

    # device-layout tiles
    def pack_idx(idx, Karr):
        tiles = np.zeros((NCORES, P, 8 * int(np.sum(Karr))), np.int16)
        for c in range(NCORES):
            cols = []
            a = idx[c].reshape(BANDS, P, -1)
            for pos in range(BANDS):
                kq = int(Karr[pos])
                lst = a[pos, :, :kq].T.reshape(-1)       # j = k*128 + r
                cols.append(np.tile(lst.reshape(-1, 16).T, (8, 1)))
            tiles[c] = np.concatenate(cols, axis=1).astype(np.int16)
        return tiles

    def pack_w(wv, Karr):
        tiles = np.zeros((NCORES, P, int(np.sum(Karr))), np.float32)
        for c in range(NCORES):
            a = wv[c].reshape(BANDS, P, -1)
            col = 0
            for pos in range(BANDS):
                kq = int(Karr[pos])
                tiles[c][:, col:col + kq] = a[pos, :, :kq]
                col += kq
        return tiles

    idx_t = [pack_idx(idxs[i], Ks[i]) for i in range(NB)]
    # combined w: per band pos, columns [k0 | k1 | k2] adjacent
    Kt = Ks.sum(axis=0)                       # [BANDS] total slots per pos
    offT = np.concatenate([[0], np.cumsum(Kt)]).astype(np.int64)
    wAll = np.zeros((NCORES, P, int(Kt.sum())), np.float32)
    for c in range(NCORES):
        col = 0
        for pos in range(BANDS):
            for i in range(NB):
                kq = int(Ks[i][pos])
                wAll[c][:, col:col + kq] = wvs[i][c].reshape(BANDS, P, -1)[pos, :, :kq]
                col += kq

    diag_t = np.zeros((NCORES, P, BANDS), np.float32)
    dn = np.zeros(NP, np.float32)
    dn[new_id] = diag_old
    diag_t[:] = dn.reshape(NCORES, P, BANDS)

    return dict(new_id=new_id, Ks=Ks, offs=offs, sumKs=sumKs, bases=bases,
                idx=idx_t, wAll=wAll, Kt=Kt, offT=offT, diag=diag_t)


# --------------------------------------------------------------- device prog

def _build_program(Ks, offs, sumKs, bases, Kt, offT):
    import concourse.bacc as bacc
    import concourse.bass as bass
    import concourse.mybir as mybir
    import concourse.tile as tile
    from concourse.masks import make_identity

    f32 = mybir.dt.float32
    i16 = mybir.dt.int16
    ADD = mybir.AluOpType.add
    SUB = mybir.AluOpType.subtract
    MULT = mybir.AluOpType.mult
    AXX = mybir.AxisListType.X

    nc = bacc.Bacc(num_devices=NCORES, target_bir_lowering=False)

    xT_in = nc.dram_tensor("xT", [P, SLOTS], f32, kind="ExternalInput")
    W1r_in = nc.dram_tensor("W1r", [P, 4 * H], f32, kind="ExternalInput")
    W2r_in = nc.dram_tensor("W2r", [H, 4 * C], f32, kind="ExternalInput")
    b1_in = nc.dram_tensor("bias1", [P, H], f32, kind="ExternalInput")
    b2_in = nc.dram_tensor("bias2", [P, C], f32, kind="ExternalInput")
    idx_ins = [nc.dram_tensor(f"idx{i}", [P, 8 * int(sumKs[i])], i16,
                              kind="ExternalInput") for i in range(3)]
    wAll_in = nc.dram_tensor("wAll", [P, int(Kt.sum())], f32,
                             kind="ExternalInput")
    diag_in = nc.dram_tensor("diag", [P, BANDS], f32, kind="ExternalInput")
    out_ext = nc.dram_tensor("out", [P, BANDS * C], f32, kind="ExternalOutput")

    vcur = nc.dram_tensor("vcur", [NP, H], f32, addr_space="Shared")
    ybounce = nc.dram_tensor("ybounce", [P, BANDS * H], f32)
    dbg = (nc.dram_tensor("dbg", [P, BANDS * H], f32, kind="ExternalOutput")
           if DEBUG_STAGE else None)

    RG = [list(range(NCORES))]

    with tile.TileContext(nc) as tc:
        with (
            tc.tile_pool(name="const", bufs=1) as cp,
            tc.tile_pool(name="work", bufs=2) as wp,
            tc.tile_pool(name="small", bufs=4) as sp,
            tc.tile_pool(name="psum", bufs=2, space="PSUM") as pp,
        ):
            # ---- resident loads
            xT = cp.tile([P, SLOTS], f32)
            nc.sync.dma_start(xT[:], xT_in[:])
            W1r = cp.tile([P, 4 * H], f32)
            nc.sync.dma_start(W1r[:], W1r_in[:])
            W2r = cp.tile([H, 4 * C], f32)
            nc.sync.dma_start(W2r[:], W2r_in[:])
            bias1 = cp.tile([P, H], f32)
            nc.sync.dma_start(bias1[:], b1_in[:])
            bias2 = cp.tile([P, C], f32)
            nc.sync.dma_start(bias2[:], b2_in[:])
            idx_ts = []
            for i in range(3):
                it = cp.tile([P, 8 * int(sumKs[i])], i16, tag=f"idx{i}")
                nc.sync.dma_start(it[:], idx_ins[i][:])
                idx_ts.append(it)
            wAll_t = cp.tile([P, int(Kt.sum())], f32, tag="wAll")
            nc.sync.dma_start(wAll_t[:], wAll_in[:])
            diag = cp.tile([P, BANDS], f32)
            nc.sync.dma_start(diag[:], diag_in[:])
            ident = cp.tile([P, P], f32)
            make_identity(nc, ident)

            b0 = cp.tile([P, BANDS * H], f32, tag="b0")
            b1t = cp.tile([P, BANDS * H], f32, tag="b1")
            b2t = cp.tile([P, BANDS * H], f32, tag="b2")
            hT = cp.tile([H, SLOTS], f32, tag="hT")
            outb = cp.tile([P, BANDS * C], f32, tag="outb")
            nc.vector.memset(outb[:], 0.0)

            def bsl(t, pos, dd=H):
                return t[:, pos * H:pos * H + dd]

            def u_mm(pos, layer, k, dd):
                """u_k band on PSUM: layer 1 from xT/W1r, layer 2 from hT/W2r."""
                ups = pp.tile([P, dd], f32, tag="u", space="PSUM")
                if layer == 1:
                    nc.tensor.matmul(
                        ups[:], lhsT=xT[:, pos * P:(pos + 1) * P],
                        rhs=W1r[:, k * H:(k + 1) * H], start=True, stop=True)
                else:
                    nc.tensor.matmul(
                        ups[:], lhsT=hT[:, pos * P:(pos + 1) * P],
                        rhs=W2r[:, k * C:(k + 1) * C], start=True, stop=True)
                return ups

            def publish(bsrc):
                """b buffer -> ybounce -> AllGather -> vcur."""
                nc.sync.dma_start(ybounce[:], bsrc[:])
                nc.gpsimd.collective_compute(
                    "AllGather", mybir.AluOpType.bypass, replica_groups=RG,
                    ins=[ybounce[:].opt()], outs=[vcur[:].opt()])

            def spmm_y(pos, dd):
                """y = scatter-part of (L_hat @ v) for band pos; returns [P, dd]."""
                if "gather" in DEBUG_SKIP:
                    y = sp.tile([P, dd], f32, tag="y")
                    nc.vector.memset(y[:], 0.0)
                    return y
                kt = int(Kt[pos])
                g = wp.tile([P, kt, H], f32, tag="g")
                col = 0
                for i in range(3):
                    kq = int(Ks[i][pos])
                    if kq == 0:
                        continue
                    nc.gpsimd.dma_gather(
                        out_ap=g[:, col:col + kq, :],
                        in_ap=vcur[bases[i]:bases[i] + IDX_CAP, :],
                        idxs_ap=idx_ts[i][:, 8 * int(offs[i][pos]):
                                          8 * int(offs[i][pos] + kq)],
                        num_idxs=P * kq, num_idxs_reg=P * kq, elem_size=H,
                        single_packet=False)
                    col += kq
                nc.vector.tensor_tensor(
                    g[:, :, :dd], g[:, :, :dd],
                    wAll_t[:, int(offT[pos]):int(offT[pos] + kt)]
                    .unsqueeze(2).to_broadcast([P, kt, dd]), op=MULT)
                y = sp.tile([P, dd], f32, tag="y")
                nc.vector.tensor_reduce(
                    y[:], g[:, :, :dd].transpose([0, 2, 1]), axis=AXX, op=ADD)
                return y

            def prop_phase(mode, layer, k, dd, bv, bdst, bprev2):
                """One Clenshaw prop: bdst = 2(L v) + u_k [- bprev2]  or the
                final combine (mode 'fin')."""
                for pos in range(BANDS):
                    y = spmm_y(pos, dd)
                    if "stt" in DEBUG_SKIP:
                        t = y
                    else:
                        t = sp.tile([P, dd], f32, tag="t")
                        nc.vector.scalar_tensor_tensor(
                            out=t[:], in0=bsl(bv, pos, dd), scalar=diag[:, pos:pos + 1],
                            in1=y[:], op0=MULT, op1=ADD)
                    if "u" in DEBUG_SKIP:
                        ups = sp.tile([P, dd], f32, tag="ustub")
                        nc.vector.memset(ups[:], 0.0)
                    else:
                        ups = u_mm(pos, layer, k, dd)
                    if mode == "b":          # 2t + u [- bprev2]
                        s = sp.tile([P, dd], f32, tag="s")
                        nc.vector.scalar_tensor_tensor(
                            out=s[:], in0=t[:], scalar=2.0, in1=ups[:],
                            op0=MULT, op1=ADD)
                        if bprev2 is not None:
                            nc.vector.tensor_sub(
                                bsl(bdst, pos, dd), s[:], bsl(bprev2, pos, dd))
                        else:
                            nc.vector.tensor_copy(bsl(bdst, pos, dd), s[:])
                    else:                    # fin: t - bprev2 + u + bias
                        s = sp.tile([P, dd], f32, tag="s")
                        nc.vector.tensor_sub(s[:], t[:], bsl(bprev2, pos, dd))
                        nc.vector.tensor_add(s[:], s[:], ups[:])
                        if layer == 1:
                            nc.vector.tensor_add(s[:], s[:], bias1[:, :dd])
                            h = bsl(bdst, pos, dd)
                            nc.vector.tensor_relu(h, s[:])
                            trp = pp.tile([H, P], f32, tag="tr", space="PSUM")
                            nc.tensor.transpose(out=trp[:], in_=h, identity=ident[:])
                            nc.scalar.copy(hT[:, pos * P:(pos + 1) * P], trp[:])
                        else:
                            nc.vector.tensor_add(s[:], s[:], bias2[:, :dd])
                            nc.vector.tensor_copy(
                                outb[:, pos * C:pos * C + C], s[:])

            def dump(stage, buf):
                if DEBUG_STAGE == stage:
                    nc.sync.dma_start(dbg[:], buf[:])

            def zero_tails():
                # zero the 40:64 columns of the b buffers for the narrow layer
                for bb in (b0, b1t, b2t):
                    nc.vector.memset(
                        bb[:].rearrange("p (b h) -> p b h", h=H)[:, :, C:H], 0.0)

            def u_loop(layer, k, dd, bdst):
                for pos in range(BANDS):
                    ups = u_mm(pos, layer, k, dd)
                    nc.vector.tensor_copy(bsl(bdst, pos, dd), ups[:])

            stages = [
                # ---------------- layer 1 ----------------
                ("u3", lambda: u_loop(1, 3, H, b0)),           # b3 = u3
                ("pub0", lambda: (publish(b0), dump("b3", b0))),
                ("b2", lambda: prop_phase("b", 1, 2, H, bv=b0, bdst=b1t,
                                          bprev2=None)),       # b2 = 2Lb3+u2
                ("pub1", lambda: (publish(b1t), dump("b2", b1t))),
                ("b1", lambda: prop_phase("b", 1, 1, H, bv=b1t, bdst=b2t,
                                          bprev2=b0)),         # b1 = 2Lb2-b3+u1
                ("pub2", lambda: (publish(b2t), dump("b1", b2t))),
                ("h", lambda: (prop_phase("fin", 1, 0, H, bv=b2t, bdst=b0,
                                          bprev2=b1t), dump("h", b0))),
                # ---------------- layer 2 ----------------
                ("zt", zero_tails),
                ("u3p", lambda: u_loop(2, 3, C, b1t)),         # b3' = u3'
                ("pub3", lambda: (publish(b1t), dump("u3p", b1t))),
                ("b2p", lambda: prop_phase("b", 2, 2, C, bv=b1t, bdst=b2t,
                                           bprev2=None)),
                ("pub4", lambda: (publish(b2t), dump("b2p", b2t))),
                ("b1p", lambda: prop_phase("b", 2, 1, C, bv=b2t, bdst=b0,
                                           bprev2=b1t)),
                ("pub5", lambda: (publish(b0), dump("b1p", b0))),
                ("fin", lambda: prop_phase("fin", 2, 0, C, bv=b0, bdst=None,
                                           bprev2=b2t)),
            ]
            for name, thunk in stages:
                thunk()
                if STOP_AFTER == name:
                    break

            nc.sync.dma_start(out_ext[:], outb[:])

    nc.compile()
    return nc


# -------------------------------------------------------------------- kernel

def kernel(x, edge_index, edge_weight, W1, b1, W2, b2):
    from concourse.bass_utils import run_bass_kernel_spmd

    x = np.asarray(x, np.float32)
    W1 = np.asarray(W1, np.float32)
    W2 = np.asarray(W2, np.float32)
    b1 = np.asarray(b1, np.float32)
    b2 = np.asarray(b2, np.float32)

    plan = _build_plan(edge_index, edge_weight)
    new_id = plan["new_id"]

    nc = _build_program(plan["Ks"], plan["offs"], plan["sumKs"], plan["bases"],
                        plan["Kt"], plan["offT"])

    # xT per core: [128 features, SLOTS] with node (pos, r) at column pos*128+r
    xp = np.zeros((NP, F), np.float32)
    xp[new_id] = x
    # padded id = c*SLOTS + r*BANDS + pos ; column order wanted: pos*128 + r
    xc = xp.reshape(NCORES, P, BANDS, F)          # [c, r, pos, F]
    xT_cores = np.ascontiguousarray(
        xc.transpose(0, 3, 2, 1).reshape(NCORES, F, SLOTS))  # [c, F, pos*128+r]

    W1r = np.ascontiguousarray(
        np.concatenate([W1[k * F:(k + 1) * F, :] for k in range(KCH)], axis=1))
    W2r = np.ascontiguousarray(
        np.concatenate([W2[k * H:(k + 1) * H, :] for k in range(KCH)], axis=1))
    bias1 = np.tile(b1[None, :], (P, 1)).astype(np.float32)
    bias2 = np.tile(b2[None, :], (P, 1)).astype(np.float32)

    in_maps = []
    for c in range(NCORES):
        in_maps.append({
            "xT": xT_cores[c],
            "W1r": W1r, "W2r": W2r, "bias1": bias1, "bias2": bias2,
            "idx0": plan["idx"][0][c], "idx1": plan["idx"][1][c],
            "idx2": plan["idx"][2][c],
            "wAll": plan["wAll"][c],
            "diag": plan["diag"][c],
        })

    trace = bool(int(os.environ.get("CHEB_TRACE", "0")))
    import time as _time
    _t0 = _time.time()
    res = run_bass_kernel_spmd(nc, in_maps, core_ids=list(range(NCORES)),
                               trace=trace)
    LAST_RESULTS["res"] = res
    LAST_RESULTS["exec_wall_s"] = _time.time() - _t0

    outs = np.stack([np.asarray(res.results[c]["out"]) for c in range(NCORES)])
    # out tile [128, BANDS*C]: row r, cols pos*C.. ; padded id = c*SLOTS+r*BANDS+pos
    res_pad = outs.reshape(NCORES, P, BANDS, C).reshape(NP, C)
    return res_pad[new_id].astype(np.float32)
